# revision 1
# baseline (speedup 1.0000x reference)
"""nn_DWTFrontEnd Trainium2 Bass kernel (v2: PE-matmul, position-major).

3-level db4 DWT band split/reconstruction of 2048 signals x 8192.
Sharding: 256 signals per core (8 cores). Inside each core, signals are
laid out position-major ([128 positions, 256 signals] blocks) so every
conv stage becomes banded-matrix matmuls on the PE engine (fp16 data,
fp32 PSUM accumulate). Synthesis uses per-band composed operators.
DVE/ACT/Pool split the PSUM->SBUF copies; host transposes shards.
"""
import sys
for p in ("/opt/trn_rl_repo", "/root/.axon_site/_ro/trn_rl_repo"):
    if p not in sys.path:
        sys.path.append(p)

import numpy as np
import concourse.bass as bass
import concourse.mybir as mybir
import concourse.tile as tile
from concourse.tile_rust import add_dep_helper

F32 = mybir.dt.float32
F16 = mybir.dt.float16

REC_LO = np.array([0.23037781330885523, 0.7148465705525415, 0.6308807679295904,
                   -0.027983769416983849, -0.18703481171888114, 0.030841381835986965,
                   0.032883011666982945, -0.010597401784997278], dtype=np.float64)
F = 8
REC_HI = np.array([(-1.0) ** k * REC_LO[F - 1 - k] for k in range(F)], dtype=np.float64)
DEC_LO = REC_LO[::-1].copy()
DEC_HI = REC_HI[::-1].copy()

N = 8192
L1, L2, L3 = 4099, 2053, 1030
S = 256            # signals per core
P = 128            # positions per block


def _dwt_apply(X, filt):
    n = X.shape[1]
    idx = np.pad(np.arange(n), (F - 1, F - 1), mode='symmetric')[1:]
    Xe = X[:, idx]
    L = (n + 13 - F) // 2 + 1
    out = np.zeros((X.shape[0], L), dtype=X.dtype)
    for k in range(F):
        out += filt[k] * Xe[:, k:k + 2 * (L - 1) + 1:2]
    return out


def _idwt_half_apply(A, filt):
    B, L = A.shape
    n_out = 2 * L - 6
    out = np.zeros((B, n_out), dtype=A.dtype)
    j = np.arange(L)
    for k in range(F):
        n = 2 * j - k + 1
        valid = (n >= 0) & (n < n_out)
        out[:, n[valid]] += filt[k] * A[:, j[valid]]
    return out


def _synth_chain(I, steps):
    A = I
    for filt, trim_to in steps:
        A = _idwt_half_apply(A, filt)
        if trim_to is not None and A.shape[1] == trim_to + 1:
            A = A[:, :trim_to]
    return A


def _build_stages():
    """Returns (stages, weights) where weights is [nW,128,128] f16 and
    stages is a list of dicts: name, src, dst, n_out_blocks, and
    blocks[i] = list of (in_block_j, widx)."""
    f32 = np.float32
    ops = {}
    I_n = np.eye(N, dtype=f32)
    ops['A1lo'] = ('x', 'cA1', _dwt_apply(I_n, REC_LO))
    ops['A1hi'] = ('x', 'cD1', _dwt_apply(I_n, REC_HI))
    del I_n
    I1 = np.eye(L1, dtype=f32)
    ops['A2lo'] = ('cA1', 'cA2', _dwt_apply(I1, REC_LO))
    ops['A2hi'] = ('cA1', 'cD2', _dwt_apply(I1, REC_HI))
    I2 = np.eye(L2, dtype=f32)
    ops['A3lo'] = ('cA2', 'cA3', _dwt_apply(I2, REC_LO))
    ops['A3hi'] = ('cA2', 'cD3', _dwt_apply(I2, REC_HI))
    ops['T3'] = ('cD1', 'y3', _synth_chain(I1, [(DEC_HI, None)])[:, :N])
    ops['T2'] = ('cD2', 'y2', _synth_chain(np.eye(L2, dtype=f32),
                                           [(DEC_HI, L1), (DEC_LO, None)])[:, :N])
    I3 = np.eye(L3, dtype=f32)
    ops['T1'] = ('cD3', 'y1', _synth_chain(I3, [(DEC_HI, L2), (DEC_LO, L1),
                                                (DEC_LO, None)])[:, :N])
    ops['T0'] = ('cA3', 'y0', _synth_chain(I3, [(DEC_LO, L2), (DEC_LO, L1),
                                                (DEC_LO, None)])[:, :N])
    del I1, I2, I3

    wmap = {}
    wlist = []
    stages = []
    for name, (src, dst, T_T) in ops.items():
        inlen, outlen = T_T.shape
        ib, ob = -(-inlen // P), -(-outlen // P)
        Tp = np.zeros((ib * P, ob * P), dtype=f32)
        Tp[:inlen, :outlen] = T_T
        nzb = np.abs(Tp.reshape(ib, P, ob, P)).max(axis=(1, 3)) > 0
        blocks = []
        for i in range(ob):
            lst = []
            for j in range(ib):
                if not nzb[j, i]:
                    continue
                W = Tp[j * P:(j + 1) * P, i * P:(i + 1) * P].astype(np.float16)
                key = W.tobytes()
                if key not in wmap:
                    wmap[key] = len(wlist)
                    wlist.append(W)
                lst.append((j, wmap[key]))
            blocks.append(lst)
        stages.append(dict(name=name, src=src, dst=dst, blocks=blocks))
    weights = np.stack(wlist)  # [nW, 128, 128]
    return stages, weights


_STAGES = None
_WEIGHTS = None


def _get_stages():
    global _STAGES, _WEIGHTS
    if _STAGES is None:
        _STAGES, _WEIGHTS = _build_stages()
    return _STAGES, _WEIGHTS


# coefficient tile sizes in blocks
NB = {'x': 64, 'cA1': 33, 'cD1': 33, 'cA2': 17, 'cD2': 17, 'cA3': 9, 'cD3': 9}
BAND_IDX = {'y0': 0, 'y1': 1, 'y2': 2, 'y3': 3}
GRP = 8            # output blocks per DMA group


def build_kernel():
    stages, weights = _get_stages()
    nW = weights.shape[0]
    # weights as [128, nW*128] (partition-major: W[k, m] at part k, col widx*128+m)
    wflat = np.ascontiguousarray(weights.transpose(1, 0, 2).reshape(P, nW * P)).astype(np.float16)

    nc = bass.Bass(trn_type="TRN2")
    x_d = nc.dram_tensor("x", [N, S], F16, kind="ExternalInput").ap()
    y_d = nc.dram_tensor("y", [4, N, S], F16, kind="ExternalOutput").ap()
    w_d = nc.inline_tensor(wflat, name="wts").ap()

    sinks = []
    eng_last = {}
    eng_cost = {'vector': 0.0, 'scalar': 0.0}
    eng_rate = {'vector': 1.05, 'scalar': 0.85}
    eng_fix = {'vector': 170.0, 'scalar': 220.0}

    with tile.TileContext(nc) as tc:
        with tc.tile_pool(name="ded", bufs=1) as ded, \
             tc.tile_pool(name="stg", bufs=8) as stg, \
             tc.tile_pool(name="psum", bufs=7, space="PSUM") as psum:

            wsb = ded.tile([P, nW * P], F16, tag="wsb")
            sinks.append(nc.sync.dma_start(wsb, w_d))

            tiles = {}
            for nm, nb in NB.items():
                tiles[nm] = ded.tile([P, nb * S], F16, tag=nm, name=nm)
            # load x: 8 chunks of 8 blocks each
            for g in range(8):
                src = x_d[g * 1024:(g + 1) * 1024, :].rearrange(
                    "(b p) s -> p b s", p=P)
                dst = tiles['x'][:, g * 8 * S:(g + 1) * 8 * S].rearrange(
                    "p (b s) -> p b s", s=S)
                sinks.append(nc.sync.dma_start(dst, src))

            def pick_engine():
                return min(eng_cost, key=eng_cost.get)

            def do_copy(dst_ap, src_ap, width):
                e = pick_engine()
                eng_cost[e] += width * eng_rate[e] + eng_fix[e]
                if e == 'vector':
                    eng_last[e] = nc.vector.tensor_copy(dst_ap, src_ap)
                elif e == 'gpsimd':
                    eng_last[e] = nc.gpsimd.tensor_copy(dst_ap, src_ap)
                else:
                    eng_last[e] = nc.scalar.mul(dst_ap, src_ap, 1.0)

            order = ['A1lo', 'A1hi', 'T3', 'A2lo', 'A2hi', 'T2',
                     'A3lo', 'A3hi', 'T1', 'T0']
            smap = {s3['name']: s3 for s3 in stages}
            for st in [smap[o] for o in order]:
                src_t = tiles[st['src']]
                is_band = st['dst'] in BAND_IDX
                blocks = st['blocks']
                ob = len(blocks)
                if is_band:
                    band = BAND_IDX[st['dst']]
                    for g in range(ob // GRP):
                        stgt = stg.tile([P, GRP * S], F16, tag="stg", name="stgt")
                        for t in range(GRP // 2):
                            ps = psum.tile([P, 2 * S], F32, tag="ps", name="ps")
                            for h in range(2):
                                i = g * GRP + 2 * t + h
                                lst = blocks[i]
                                for q, (j, widx) in enumerate(lst):
                                    nc.tensor.matmul(
                                        ps[:, h * S:(h + 1) * S],
                                        wsb[:, widx * P:(widx + 1) * P],
                                        src_t[:, j * S:(j + 1) * S],
                                        start=(q == 0), stop=(q == len(lst) - 1))
                            do_copy(stgt[:, 2 * t * S:(2 * t + 2) * S], ps, 2 * S)
                        dma_dst = y_d[band, g * GRP * P:(g + 1) * GRP * P, :] \
                            .rearrange("(b p) s -> p b s", p=P)
                        dma_src = stgt.rearrange("p (b s) -> p b s", s=S)
                        sinks.append(nc.sync.dma_start(dma_dst, dma_src))
                else:
                    dst_t = tiles[st['dst']]
                    for t in range(-(-ob // 2)):
                        ps = psum.tile([P, 2 * S], F32, tag="ps", name="ps")
                        nh = min(2, ob - 2 * t)
                        for h in range(nh):
                            i = 2 * t + h
                            lst = blocks[i]
                            for q, (j, widx) in enumerate(lst):
                                nc.tensor.matmul(
                                    ps[:, h * S:(h + 1) * S],
                                    wsb[:, widx * P:(widx + 1) * P],
                                    src_t[:, j * S:(j + 1) * S],
                                    start=(q == 0), stop=(q == len(lst) - 1))
                        do_copy(dst_t[:, 2 * t * S:(2 * t + nh) * S],
                                ps[:, :nh * S], nh * S)

            tc.no_sync_barrier()
            for s2 in sinks + list(eng_last.values()):
                nn = nc.sync.nop()
                add_dep_helper(nn.ins, s2.ins, reason="tail absorb")
    # PE is hardware-decoded: a Matmult may carry at most one sync wait.
    # Move extra waits onto the paired (SW-decoded) Ldweights, then split
    # any remaining multi-wait instructions via event semaphores.
    import bass_rust
    bass_rust.move_matmul_waits_to_ldweights(nc.m)
    bass_rust.generate_event_semaphores(nc)
    return nc


_NC_CACHE = None


def run_full(x_full, trace=False):
    from concourse.bass_utils import run_bass_kernel_spmd
    global _NC_CACHE
    B, C, n = x_full.shape
    xf = np.ascontiguousarray(x_full.reshape(B * C, n).astype(np.float32))
    n_cores = 8
    if _NC_CACHE is None:
        _NC_CACHE = build_kernel()
    nc = _NC_CACHE
    in_maps = []
    for i in range(n_cores):
        shard = xf[i * S:(i + 1) * S]                    # (256, 8192)
        xt = np.ascontiguousarray(shard.T.astype(np.float16))  # (8192, 256)
        in_maps.append({"x": xt})
    res = run_bass_kernel_spmd(nc, in_maps, core_ids=list(range(n_cores)),
                               trace=trace)
    bands = np.empty((4, B * C, n), dtype=np.float32)
    for i in range(n_cores):
        y = np.asarray(res.results[i]["y"])              # (4, 8192, 256) f16
        bands[:, i * S:(i + 1) * S, :] = y.transpose(0, 2, 1).astype(np.float32)
    out = tuple(bands[j].reshape(B, C, n) for j in range(4))
    return out, res


def kernel(x):
    out, _ = run_full(np.asarray(x))
    return out



# revision 2
# speedup vs baseline: 1.3734x; 1.3734x over previous
"""nn_DWTFrontEnd Trainium2 Bass kernel (v3: composed band operators).

3-level db4 DWT band split/reconstruction of 2048 signals x 8192.
Sharding: 256 signals per core (8 cores), position-major SBUF layout
([128 positions, 256 signals] blocks).

v3 idea: fold analysis+synthesis per band into a single banded operator
P_b (8192x8192, support +-49 samples).  Output is computed in
half-block-SHIFTED 128-position tiles ([64+128i, 192+128i)) so each
output tile needs exactly TWO 128-aligned input K-tiles -> 2 matmuls
per tile, 128 per band, 512 total (vs 660 in the staged version), and
no intermediate coefficient tensors or copies.  Only 16 distinct
128x128 weight blocks exist (Toeplitz structure), so weight DMA is
~0.4 MiB.  fp16 data/weights, fp32 PSUM accumulate.  DVE/ACT split the
PSUM->SBUF copies; all DMAs issue from the SP queue.
"""
import sys
for p in ("/opt/trn_rl_repo", "/root/.axon_site/_ro/trn_rl_repo"):
    if p not in sys.path:
        sys.path.append(p)

import numpy as np
import concourse.bass as bass
import concourse.mybir as mybir
import concourse.tile as tile
from concourse.tile_rust import add_dep_helper

F32 = mybir.dt.float32
F16 = mybir.dt.float16

REC_LO = np.array([0.23037781330885523, 0.7148465705525415, 0.6308807679295904,
                   -0.027983769416983849, -0.18703481171888114, 0.030841381835986965,
                   0.032883011666982945, -0.010597401784997278], dtype=np.float64)
F = 8
REC_HI = np.array([(-1.0) ** k * REC_LO[F - 1 - k] for k in range(F)], dtype=np.float64)
DEC_LO = REC_LO[::-1].copy()
DEC_HI = REC_HI[::-1].copy()

N = 8192
L1, L2, L3 = 4099, 2053, 1030
S = 256            # signals per core
P = 128            # positions per block
NBLK = N // P      # 64 input blocks
NE = 65            # output entries per band: head + 63 shifted + tail


def _dwt_apply(X, filt):
    n = X.shape[1]
    idx = np.pad(np.arange(n), (F - 1, F - 1), mode='symmetric')[1:]
    Xe = X[:, idx]
    L = (n + 13 - F) // 2 + 1
    out = np.zeros((X.shape[0], L), dtype=X.dtype)
    for k in range(F):
        out += filt[k] * Xe[:, k:k + 2 * (L - 1) + 1:2]
    return out


def _idwt_half_apply(A, filt):
    B, L = A.shape
    n_out = 2 * L - 6
    out = np.zeros((B, n_out), dtype=A.dtype)
    j = np.arange(L)
    for k in range(F):
        n = 2 * j - k + 1
        valid = (n >= 0) & (n < n_out)
        out[:, n[valid]] += filt[k] * A[:, j[valid]]
    return out


def _synth_chain(I, steps):
    A = I
    for filt, trim_to in steps:
        A = _idwt_half_apply(A, filt)
        if trim_to is not None and A.shape[1] == trim_to + 1:
            A = A[:, :trim_to]
    return A


def _band_operator(b):
    """P[r, m] = d y_b[m] / d x[r], shape (N, N), f32."""
    f32 = np.float32
    if b == 0:
        T = _synth_chain(np.eye(L3, dtype=f32), [(DEC_LO, L2), (DEC_LO, L1),
                                                 (DEC_LO, None)])[:, :N]
        M = _dwt_apply(np.eye(L2, dtype=f32), REC_LO) @ T
        M = _dwt_apply(np.eye(L1, dtype=f32), REC_LO) @ M
        M = _dwt_apply(np.eye(N, dtype=f32), REC_LO) @ M
    elif b == 1:
        T = _synth_chain(np.eye(L3, dtype=f32), [(DEC_HI, L2), (DEC_LO, L1),
                                                 (DEC_LO, None)])[:, :N]
        M = _dwt_apply(np.eye(L2, dtype=f32), REC_HI) @ T
        M = _dwt_apply(np.eye(L1, dtype=f32), REC_LO) @ M
        M = _dwt_apply(np.eye(N, dtype=f32), REC_LO) @ M
    elif b == 2:
        T = _synth_chain(np.eye(L2, dtype=f32), [(DEC_HI, L1), (DEC_LO, None)])[:, :N]
        M = _dwt_apply(np.eye(L1, dtype=f32), REC_HI) @ T
        M = _dwt_apply(np.eye(N, dtype=f32), REC_LO) @ M
    else:
        T = _synth_chain(np.eye(L1, dtype=f32), [(DEC_HI, None)])[:, :N]
        M = _dwt_apply(np.eye(N, dtype=f32), REC_HI) @ T
    return M


def _build_plan():
    """entries[b] = 65 tuples (out_lo, out_hi, [(k_block, widx), ...]).
    Weight blocks are [128,128] f16 (head/tail padded with zero columns);
    out tile for entry e covers positions [out_lo, out_hi) at PSUM
    partitions 0..(out_hi-out_lo)."""
    wmap, wlist = {}, []

    def wid(W):
        Wp = np.zeros((P, P), dtype=np.float16)
        Wp[:W.shape[0], :W.shape[1]] = W.astype(np.float16)
        key = Wp.tobytes()
        if key not in wmap:
            wmap[key] = len(wlist)
            wlist.append(Wp)
        return wmap[key]

    Ps = [_band_operator(b) for b in range(4)]
    entries = [[] for _ in range(4)]
    # entry-major so early-used weights come first in wlist
    for e in range(NE):
        for b in range(4):
            Pb = Ps[b]
            if e == 0:
                lo, hi = 0, 64
                ks = [0]
            elif e == NE - 1:
                lo, hi = N - 64, N
                ks = [NBLK - 1]
            else:
                lo = 64 + P * (e - 1)
                hi = lo + P
                ks = [e - 1, e]
            # sanity: operator support outside chosen K-tiles must be zero
            mask = np.ones(N, dtype=bool)
            for kb in ks:
                mask[kb * P:(kb + 1) * P] = False
            assert np.abs(Pb[mask][:, lo:hi]).max() == 0.0, (b, e)
            mm = [(kb, wid(Pb[kb * P:(kb + 1) * P, lo:hi])) for kb in ks]
            entries[b].append((lo, hi, mm))
    weights = np.stack(wlist)  # [nW, 128, 128]
    return entries, weights


_PLAN = None


def _get_plan():
    global _PLAN
    if _PLAN is None:
        _PLAN = _build_plan()
    return _PLAN


def build_kernel():
    entries, weights = _get_plan()
    nW = weights.shape[0]
    # [128, nW*128] partition-major: W[k, m] at partition k, col widx*128+m
    wflat = np.ascontiguousarray(
        weights.transpose(1, 0, 2).reshape(P, nW * P)).astype(np.float16)

    nc = bass.Bass(trn_type="TRN2")
    x_d = nc.dram_tensor("x", [N, S], F16, kind="ExternalInput").ap()
    y_d = nc.dram_tensor("y", [4, N, S], F16, kind="ExternalOutput").ap()
    w_d = nc.inline_tensor(wflat, name="wts").ap()

    sinks = []
    eng_last = {}
    eng_cost = {'vector': 0.0, 'scalar': 0.0}
    eng_rate = {'vector': 1.05, 'scalar': 0.85}
    eng_fix = {'vector': 170.0, 'scalar': 220.0}

    with tile.TileContext(nc) as tc:
        with tc.tile_pool(name="ded", bufs=1) as ded, \
             tc.tile_pool(name="stg", bufs=6) as stg, \
             tc.tile_pool(name="psA", bufs=3, space="PSUM") as psA, \
             tc.tile_pool(name="psB", bufs=2, space="PSUM") as psB:

            wsb = ded.tile([P, nW * P], F16, tag="wsb")
            sinks.append(nc.sync.dma_start(wsb, w_d))

            x_t = ded.tile([P, NBLK * S], F16, tag="xt", name="xt")

            def load_x(b0, b1):
                src = x_d[b0 * P:b1 * P, :].rearrange("(b p) s -> p b s", p=P)
                dst = x_t[:, b0 * S:b1 * S].rearrange("p (b s) -> p b s", s=S)
                sinks.append(nc.sync.dma_start(dst, src))

            load_x(0, 2)       # tiny first chunk so PE starts early
            load_x(2, 8)
            for g in range(1, 8):
                load_x(8 * g, 8 * g + 8)

            def pick_engine():
                return min(eng_cost, key=eng_cost.get)

            def do_copy(dst_ap, src_ap, width):
                e = pick_engine()
                eng_cost[e] += width * eng_rate[e] + eng_fix[e]
                if e == 'vector':
                    eng_last[e] = nc.vector.tensor_copy(dst_ap, src_ap)
                else:
                    eng_last[e] = nc.scalar.mul(dst_ap, src_ap, 1.0)

            for g in range(8):
                es = list(range(8 * g, min(8 * g + 8, NE)))
                if g == 7:
                    es.append(NE - 1)
                ns = len(es)           # 8, or 9 for g==7
                for b in range(4):
                    bent = entries[b]
                    stage = stg.tile([P, 9 * S], F16, tag="stg", name="stg")
                    # psum tiles of 4 entries each; leftover entry solo
                    for t in range(0, ns, 4):
                        sub = es[t:t + 4]
                        if len(sub) == 4:
                            ps = psA.tile([P, 4 * S], F32, tag="psA", name="psA")
                        else:
                            ps = psB.tile([P, len(sub) * S], F32, tag="psB",
                                          name="psB")
                        for h, e in enumerate(sub):
                            lst = bent[e][2]
                            for q, (kb, widx) in enumerate(lst):
                                nc.tensor.matmul(
                                    ps[:, h * S:(h + 1) * S],
                                    wsb[:, widx * P:(widx + 1) * P],
                                    x_t[:, kb * S:(kb + 1) * S],
                                    start=(q == 0), stop=(q == len(lst) - 1))
                        do_copy(stage[:, t * S:(t + len(sub)) * S], ps,
                                len(sub) * S)
                    # DMA this group's output
                    if g == 0:
                        # head: entry 0 -> y[b, 0:64); entries 1..7 -> [64,960)
                        sinks.append(nc.sync.dma_start(
                            y_d[b, 0:64, :], stage[0:64, 0:S]))
                        dst = y_d[b, 64:960, :].rearrange("(k p) s -> p k s", p=P)
                        src = stage[:, S:8 * S].rearrange("p (k s) -> p k s", s=S)
                        sinks.append(nc.sync.dma_start(dst, src))
                    elif g == 7:
                        dst = y_d[b, 7104:8128, :].rearrange(
                            "(k p) s -> p k s", p=P)
                        src = stage[:, 0:8 * S].rearrange("p (k s) -> p k s", s=S)
                        sinks.append(nc.sync.dma_start(dst, src))
                        sinks.append(nc.sync.dma_start(
                            y_d[b, N - 64:N, :], stage[0:64, 8 * S:9 * S]))
                    else:
                        lo = 1024 * g - 64
                        dst = y_d[b, lo:lo + 1024, :].rearrange(
                            "(k p) s -> p k s", p=P)
                        src = stage[:, 0:8 * S].rearrange("p (k s) -> p k s", s=S)
                        sinks.append(nc.sync.dma_start(dst, src))

            tc.no_sync_barrier()
            for s2 in sinks + list(eng_last.values()):
                nn = nc.sync.nop()
                add_dep_helper(nn.ins, s2.ins, reason="tail absorb")
    # PE is hardware-decoded: a Matmult may carry at most one sync wait.
    # Move extra waits onto the paired (SW-decoded) Ldweights, then split
    # any remaining multi-wait instructions via event semaphores.
    import bass_rust
    bass_rust.move_matmul_waits_to_ldweights(nc.m)
    bass_rust.generate_event_semaphores(nc)
    return nc


_NC_CACHE = None


def run_full(x_full, trace=False):
    from concourse.bass_utils import run_bass_kernel_spmd
    global _NC_CACHE
    B, C, n = x_full.shape
    xf = np.ascontiguousarray(x_full.reshape(B * C, n).astype(np.float32))
    n_cores = 8
    if _NC_CACHE is None:
        _NC_CACHE = build_kernel()
    nc = _NC_CACHE
    in_maps = []
    for i in range(n_cores):
        shard = xf[i * S:(i + 1) * S]                    # (256, 8192)
        xt = np.ascontiguousarray(shard.T.astype(np.float16))  # (8192, 256)
        in_maps.append({"x": xt})
    res = run_bass_kernel_spmd(nc, in_maps, core_ids=list(range(n_cores)),
                               trace=trace)
    bands = np.empty((4, B * C, n), dtype=np.float32)
    for i in range(n_cores):
        y = np.asarray(res.results[i]["y"])              # (4, 8192, 256) f16
        bands[:, i * S:(i + 1) * S, :] = y.transpose(0, 2, 1).astype(np.float32)
    out = tuple(bands[j].reshape(B, C, n) for j in range(4))
    return out, res


def kernel(x):
    out, _ = run_full(np.asarray(x))
    return out


# revision 4
# speedup vs baseline: 1.4758x; 1.0746x over previous
"""nn_DWTFrontEnd Trainium2 Bass kernel (v4: composed band ops, fp8 DoubleRow).

3-level db4 DWT band split/reconstruction of 2048 signals x 8192.
Sharding: 256 signals per core (8 cores), position-major SBUF layout
([128 positions, 256 signals] blocks).

Each band's analysis+synthesis is folded into one banded operator P_b
(8192x8192, support +-49 samples).  Output is computed in half-block-
SHIFTED 128-position tiles so each output tile needs exactly one PAIR
of 128-aligned input K-tiles -> one fp8 DoubleRow matmul contracts both
K-tiles at 0.5 cycles/row.  Precision is recovered with a 3-term
expansion (weights scaled by SW=8 so fp8 residuals stay out of the
subnormal floor; the 1/SW rescale rides the PSUM->SBUF copies):

    psum = Wh (x) xh  +  Wh (x) xl  +  Wl (x) xh        (all fp8e4 DR)
    y    = psum / SW                                     (copy w/ scale)

with xh = fp8(x), xl = fp8(x - xh) split on the host (same input bytes
as fp16), Wh = fp8(SW*W), Wl = fp8(SW*W - Wh).  Weight pairs dedupe to
6 per band (Toeplitz) -> 768 KiB.  Verified vs reference: rel ~2.4e-3.
DVE/ACT split the scaled copies; all DMAs issue from the SP queue.
"""
import sys
for p in ("/opt/trn_rl_repo", "/root/.axon_site/_ro/trn_rl_repo"):
    if p not in sys.path:
        sys.path.append(p)

import numpy as np
import ml_dtypes
import concourse.bass as bass
import concourse.mybir as mybir
import concourse.tile as tile
from concourse.tile_rust import add_dep_helper

F32 = mybir.dt.float32
F16 = mybir.dt.float16
F8 = mybir.dt.float8e4
E4 = ml_dtypes.float8_e4m3
DR = mybir.MatmulPerfMode.DoubleRow

REC_LO = np.array([0.23037781330885523, 0.7148465705525415, 0.6308807679295904,
                   -0.027983769416983849, -0.18703481171888114, 0.030841381835986965,
                   0.032883011666982945, -0.010597401784997278], dtype=np.float64)
F = 8
REC_HI = np.array([(-1.0) ** k * REC_LO[F - 1 - k] for k in range(F)], dtype=np.float64)
DEC_LO = REC_LO[::-1].copy()
DEC_HI = REC_HI[::-1].copy()

N = 8192
L1, L2, L3 = 4099, 2053, 1030
S = 256            # signals per core
P = 128            # positions per block
NBLK = N // P      # 64 input blocks
NE = 65            # output entries per band: head + 63 shifted + tail
SW = 8.0           # weight pre-scale (undone in the copy)


def _dwt_apply(X, filt):
    n = X.shape[1]
    idx = np.pad(np.arange(n), (F - 1, F - 1), mode='symmetric')[1:]
    Xe = X[:, idx]
    L = (n + 13 - F) // 2 + 1
    out = np.zeros((X.shape[0], L), dtype=X.dtype)
    for k in range(F):
        out += filt[k] * Xe[:, k:k + 2 * (L - 1) + 1:2]
    return out


def _idwt_half_apply(A, filt):
    B, L = A.shape
    n_out = 2 * L - 6
    out = np.zeros((B, n_out), dtype=A.dtype)
    j = np.arange(L)
    for k in range(F):
        n = 2 * j - k + 1
        valid = (n >= 0) & (n < n_out)
        out[:, n[valid]] += filt[k] * A[:, j[valid]]
    return out


def _synth_chain(I, steps):
    A = I
    for filt, trim_to in steps:
        A = _idwt_half_apply(A, filt)
        if trim_to is not None and A.shape[1] == trim_to + 1:
            A = A[:, :trim_to]
    return A


def _band_operator(b):
    """P[r, m] = d y_b[m] / d x[r], shape (N, N), f32."""
    f32 = np.float32
    if b == 0:
        T = _synth_chain(np.eye(L3, dtype=f32), [(DEC_LO, L2), (DEC_LO, L1),
                                                 (DEC_LO, None)])[:, :N]
        M = _dwt_apply(np.eye(L2, dtype=f32), REC_LO) @ T
        M = _dwt_apply(np.eye(L1, dtype=f32), REC_LO) @ M
        M = _dwt_apply(np.eye(N, dtype=f32), REC_LO) @ M
    elif b == 1:
        T = _synth_chain(np.eye(L3, dtype=f32), [(DEC_HI, L2), (DEC_LO, L1),
                                                 (DEC_LO, None)])[:, :N]
        M = _dwt_apply(np.eye(L2, dtype=f32), REC_HI) @ T
        M = _dwt_apply(np.eye(L1, dtype=f32), REC_LO) @ M
        M = _dwt_apply(np.eye(N, dtype=f32), REC_LO) @ M
    elif b == 2:
        T = _synth_chain(np.eye(L2, dtype=f32), [(DEC_HI, L1), (DEC_LO, None)])[:, :N]
        M = _dwt_apply(np.eye(L1, dtype=f32), REC_HI) @ T
        M = _dwt_apply(np.eye(N, dtype=f32), REC_LO) @ M
    else:
        T = _synth_chain(np.eye(L1, dtype=f32), [(DEC_HI, None)])[:, :N]
        M = _dwt_apply(np.eye(N, dtype=f32), REC_HI) @ T
    return M


def _q8(a):
    return a.astype(E4).astype(np.float32)


def _build_plan():
    """Per band b: entries[b] = 65 tuples (kstart, [(pairidx, hl), x3]).
    Entry e covers output positions: e==0 -> [0,64); e==64 -> [8128,8192);
    else [64+128(e-1), 64+128e).  K-tile pair = input blocks
    (kstart, kstart+1).  hl selects the xh (0) or xl (1) half of the
    interleaved fp8 input.  wpairs: [nP, 2, 128, 128] fp8-valued f32."""
    wpairs = []

    def addpair(WA, WB):
        wpairs.append(np.stack([WA, WB]))
        return len(wpairs) - 1

    entries = []
    for b in range(4):
        Pb = _band_operator(b)
        # verify Toeplitz interior + banded support assumptions
        H = np.zeros((P, P), np.float32)
        H[:, :64] = SW * Pb[0:P, 0:64]
        L = SW * Pb[0:P, 64:192]
        R = SW * Pb[P:2 * P, 64:192]
        T = np.zeros((P, P), np.float32)
        T[:, :64] = SW * Pb[63 * P:64 * P, N - 64:N]
        assert np.abs(Pb[2 * P:, 0:64]).max() < 1e-8
        assert np.abs(Pb[:61 * P, N - 64:N]).max() < 1e-8
        for e in range(1, 64):
            lo = 64 + P * (e - 1)
            assert np.allclose(SW * Pb[(e - 1) * P:e * P, lo:lo + P], L,
                               rtol=0, atol=1e-4)
            assert np.allclose(SW * Pb[e * P:(e + 1) * P, lo:lo + P], R,
                               rtol=0, atol=1e-4)
            mask = np.ones(N, dtype=bool)
            mask[(e - 1) * P:(e + 1) * P] = False
            assert np.abs(Pb[mask][:, lo:lo + P]).max() < 1e-8
        Z = np.zeros((P, P), np.float32)
        Hh, Lh, Rh, Th = _q8(H), _q8(L), _q8(R), _q8(T)
        Hl, Ll, Rl, Tl = _q8(H - Hh), _q8(L - Lh), _q8(R - Rh), _q8(T - Th)
        pHh = addpair(Hh, Z)
        pHl = addpair(Hl, Z)
        pIh = addpair(Lh, Rh)
        pIl = addpair(Ll, Rl)
        pTh = addpair(Z, Th)
        pTl = addpair(Z, Tl)
        bent = []
        for e in range(NE):
            if e == 0:
                ks, (ph, pl) = 0, (pHh, pHl)
            elif e == NE - 1:
                ks, (ph, pl) = NBLK - 2, (pTh, pTl)
            else:
                ks, (ph, pl) = e - 1, (pIh, pIl)
            bent.append((ks, [(ph, 0), (ph, 1), (pl, 0)]))
        entries.append(bent)
    return entries, np.stack(wpairs)  # [nP, 2, 128, 128]


_PLAN = None


def _get_plan():
    global _PLAN
    if _PLAN is None:
        _PLAN = _build_plan()
    return _PLAN


def build_kernel():
    entries, wpairs = _get_plan()
    nP = wpairs.shape[0]
    # [128, nP*2*128] fp8: col = pair*256 + two*128 + m (partition = K)
    wflat = np.ascontiguousarray(
        wpairs.transpose(2, 0, 1, 3).reshape(P, nP * 2 * P)).astype(E4)

    nc = bass.Bass(trn_type="TRN2")
    # x8: [position, {hi,lo}, signal] so the innermost run stays 512B
    x_d = nc.dram_tensor("x", [N, 2, S], F8, kind="ExternalInput").ap()
    y_d = nc.dram_tensor("y", [4, N, S], F16, kind="ExternalOutput").ap()
    w_d = nc.inline_tensor(wflat, name="wts").ap()

    sinks = []
    eng_last = {}
    eng_cost = {'vector': 0.0, 'scalar': 0.0}
    eng_rate = {'vector': 1.05, 'scalar': 0.85}
    eng_fix = {'vector': 170.0, 'scalar': 220.0}
    inv = 1.0 / SW

    with tile.TileContext(nc) as tc:
        with tc.tile_pool(name="ded", bufs=1) as ded, \
             tc.tile_pool(name="stg", bufs=6) as stg, \
             tc.tile_pool(name="psA", bufs=3, space="PSUM") as psA, \
             tc.tile_pool(name="psB", bufs=2, space="PSUM") as psB:

            wsb = ded.tile([P, nP * 2 * P], F8, tag="wsb")
            sinks.append(nc.sync.dma_start(wsb, w_d))
            w4 = wsb.rearrange("p (n two f) -> p n two f", two=2, f=P)

            x_t = ded.tile([P, NBLK * 2 * S], F8, tag="xt", name="xt")
            x4 = x_t.rearrange("p (b hl s) -> p b hl s", hl=2, s=S)

            def load_x(b0, b1):
                src = x_d[b0 * P:b1 * P, :, :].rearrange(
                    "(b p) hl s -> p b hl s", p=P)
                dst = x4[:, b0:b1]
                sinks.append(nc.sync.dma_start(dst, src))

            load_x(0, 2)       # tiny first chunk so PE starts early
            load_x(2, 8)
            for g in range(1, 8):
                load_x(8 * g, 8 * g + 8)

            def pick_engine():
                return min(eng_cost, key=eng_cost.get)

            def do_copy(dst_ap, src_ap, width):
                e = pick_engine()
                eng_cost[e] += width * eng_rate[e] + eng_fix[e]
                if e == 'vector':
                    eng_last[e] = nc.vector.tensor_scalar_mul(dst_ap, src_ap, inv)
                else:
                    eng_last[e] = nc.scalar.mul(dst_ap, src_ap, inv)

            for g in range(8):
                es = list(range(8 * g, min(8 * g + 8, NE)))
                if g == 7:
                    es.append(NE - 1)
                ns = len(es)           # 8, or 9 for g==7
                for b in range(4):
                    bent = entries[b]
                    stage = stg.tile([P, 9 * S], F16, tag="stg", name="stg")
                    for t in range(0, ns, 4):
                        sub = es[t:t + 4]
                        if len(sub) == 4:
                            ps = psA.tile([P, 4 * S], F32, tag="psA", name="psA")
                        else:
                            ps = psB.tile([P, len(sub) * S], F32, tag="psB",
                                          name="psB")
                        for h, e in enumerate(sub):
                            ks, terms = bent[e]
                            for q, (pidx, hl) in enumerate(terms):
                                nc.tensor.matmul(
                                    ps[:, h * S:(h + 1) * S],
                                    w4[:, pidx],
                                    x4[:, ks:ks + 2, hl],
                                    start=(q == 0), stop=(q == len(terms) - 1),
                                    perf_mode=DR)
                        do_copy(stage[:, t * S:(t + len(sub)) * S], ps,
                                len(sub) * S)
                    # DMA this group's output
                    if g == 0:
                        sinks.append(nc.sync.dma_start(
                            y_d[b, 0:64, :], stage[0:64, 0:S]))
                        dst = y_d[b, 64:960, :].rearrange("(k p) s -> p k s", p=P)
                        src = stage[:, S:8 * S].rearrange("p (k s) -> p k s", s=S)
                        sinks.append(nc.sync.dma_start(dst, src))
                    elif g == 7:
                        dst = y_d[b, 7104:8128, :].rearrange(
                            "(k p) s -> p k s", p=P)
                        src = stage[:, 0:8 * S].rearrange("p (k s) -> p k s", s=S)
                        sinks.append(nc.sync.dma_start(dst, src))
                        sinks.append(nc.sync.dma_start(
                            y_d[b, N - 64:N, :], stage[0:64, 8 * S:9 * S]))
                    else:
                        lo = 1024 * g - 64
                        dst = y_d[b, lo:lo + 1024, :].rearrange(
                            "(k p) s -> p k s", p=P)
                        src = stage[:, 0:8 * S].rearrange("p (k s) -> p k s", s=S)
                        sinks.append(nc.sync.dma_start(dst, src))

            tc.no_sync_barrier()
            for s2 in sinks + list(eng_last.values()):
                nn = nc.sync.nop()
                add_dep_helper(nn.ins, s2.ins, reason="tail absorb")
    # PE is hardware-decoded: a Matmult may carry at most one sync wait.
    # Move extra waits onto the paired (SW-decoded) Ldweights, then split
    # any remaining multi-wait instructions via event semaphores.
    import bass_rust
    bass_rust.move_matmul_waits_to_ldweights(nc.m)
    bass_rust.generate_event_semaphores(nc)
    return nc


_NC_CACHE = None


def run_full(x_full, trace=False):
    from concourse.bass_utils import run_bass_kernel_spmd
    global _NC_CACHE
    B, C, n = x_full.shape
    xf = np.ascontiguousarray(x_full.reshape(B * C, n).astype(np.float32))
    n_cores = 8
    if _NC_CACHE is None:
        _NC_CACHE = build_kernel()
    nc = _NC_CACHE
    in_maps = []
    for i in range(n_cores):
        shard = xf[i * S:(i + 1) * S].T                  # (8192, 256) f32
        xh = shard.astype(E4)
        xl = (shard - xh.astype(np.float32)).astype(E4)
        x8 = np.ascontiguousarray(
            np.stack([xh, xl], axis=1))                  # (8192, 2, 256) fp8
        in_maps.append({"x": x8})
    res = run_bass_kernel_spmd(nc, in_maps, core_ids=list(range(n_cores)),
                               trace=trace)
    bands = np.empty((4, B * C, n), dtype=np.float32)
    for i in range(n_cores):
        y = np.asarray(res.results[i]["y"])              # (4, 8192, 256) f16
        bands[:, i * S:(i + 1) * S, :] = y.transpose(0, 2, 1).astype(np.float32)
    out = tuple(bands[j].reshape(B, C, n) for j in range(4))
    return out, res


def kernel(x):
    out, _ = run_full(np.asarray(x))
    return out
